# revision 23
# baseline (speedup 1.0000x reference)
"""Trainium2 Bass kernel for ConvMultiHeadAttention (N=16, L=1024, E=512, H=8).

Data-parallel over batch: 8 NeuronCores x 2 batches each.

v2 design (vs v1 baseline at 313us):
  - Host marshalling: q/k/v pre-cast to fp16 (halves input DMA bytes, kills
    48 device CAST instrs), weights pre-transposed + pre-cast + wq pre-scaled
    on host (kills 64 PE transposes + 16 psum copies + 4MB of DMA).
  - Attention pipelined at (lt, ch) granularity: S pair matmuls write a
    [P, 2, 512] fp32 psum tile, ONE exp instr per unit covers both heads.
    AVch0 accumulation lags ~1 lt behind exp via an explicit high-priority
    AV queue; AVch1 re-reads kept pts as coarse filler units.
  - vh stationary is ones-extended to 128 columns: the AV matmul replicates
    the softmax denominator across psum partitions 64..127 for free (M does
    not affect matmul time). reciprocal_approx_fast reads it straight from
    PSUM partition-parallel, and a fused scalar_tensor_tensor writes the
    normalized stage in one DVE op. No [1,512] denominator copies, no
    selector matmuls, no cross-pair recip gating.
  - Integral-controller pump: tracks emitted attention-PE-ns vs exp-ns and
    feeds prep/proj/outproj filler units into the PE stream at the deficit
    rate, so the queue lasts through BOTH batches' attention phases.
  - Exp activation table preloaded by a dummy exp during the DMA fill.
"""

import numpy as np
import concourse.bass as bass
import concourse.mybir as mybir
import concourse.tile as tile
from contextlib import ExitStack
from concourse import bacc

P = 128
L = 1024
E = 512
H = 8
D = 64
NB = 2            # batches per core
TT = L // P       # 8 token tiles per batch
EPO = E // P      # 4 e-subtiles
NCH = L // E      # 2 q-chunks
FP32 = mybir.dt.float32
FP16 = mybir.dt.float16
AF = mybir.ActivationFunctionType
ALU = mybir.AluOpType


def build(debug=False):
    nc = bacc.Bacc("TRN2", target_bir_lowering=False, debug=debug)
    q_d = nc.dram_tensor("qT", [NB, P, EPO, L], FP16, kind="ExternalInput").ap()
    k_d = nc.dram_tensor("kT", [NB, P, EPO, L], FP16, kind="ExternalInput").ap()
    v_d = nc.dram_tensor("vT", [NB, P, EPO, L], FP16, kind="ExternalInput").ap()
    wt_d = {
        w: nc.dram_tensor(f"Wt_{w}", [P, EPO, E], FP16, kind="ExternalInput").ap()
        for w in ("q", "k", "v", "o")
    }
    bo_d = nc.dram_tensor("bo_row", [1, E], FP16, kind="ExternalInput").ap()
    out_d = nc.dram_tensor("outT", [NB, E, L], FP32, kind="ExternalOutput").ap()
    x_d = {"q": q_d, "k": k_d, "v": v_d}

    with tile.TileContext(nc) as tc, ExitStack() as ctx:
        consts = ctx.enter_context(tc.tile_pool(name="consts", bufs=1))
        wt_pool = ctx.enter_context(tc.tile_pool(name="wt", bufs=1))
        xt_pool = ctx.enter_context(tc.tile_pool(name="xt", bufs=2))
        qk_pool = ctx.enter_context(tc.tile_pool(name="qk", bufs=2))
        vh_pool = ctx.enter_context(tc.tile_pool(name="vh", bufs=2))
        st_pool = ctx.enter_context(tc.tile_pool(name="st", bufs=2))
        p0_pool = ctx.enter_context(tc.tile_pool(name="p0", bufs=8))
        p1_pool = ctx.enter_context(tc.tile_pool(name="p1", bufs=12))
        rc_pool = ctx.enter_context(tc.tile_pool(name="rc", bufs=2))
        o_pool = ctx.enter_context(tc.tile_pool(name="oo", bufs=4))
        scr_pool = ctx.enter_context(tc.tile_pool(name="scr", bufs=1))
        ps_s = ctx.enter_context(tc.tile_pool(name="pss", bufs=2, space="PSUM"))
        ps_av = ctx.enter_context(tc.tile_pool(name="psav", bufs=2, space="PSUM"))
        ps_mm = ctx.enter_context(tc.tile_pool(name="psmm", bufs=2, space="PSUM"))

        # ---- constants (sync queue) ----
        wts = {}
        for w in ("k", "q"):
            wt = wt_pool.tile([P, EPO, E], FP16, tag=f"wt_{w}", name=f"wt_{w}")
            nc.sync.dma_start(wt[:], wt_d[w])
            wts[w] = wt

        def emit_wt(w):
            wt = wt_pool.tile([P, EPO, E], FP16, tag=f"wt_{w}", name=f"wt_{w}")
            nc.sync.dma_start(wt[:], wt_d[w])
            wts[w] = wt

        bo_t = consts.tile([1, E], FP16)
        nc.sync.dma_start(bo_t[:], bo_d)
        ones_row = consts.tile([1, E], FP16)
        nc.gpsimd.memset(ones_row[:], 1.0)

        # preload the Exp table while DMAs stream in
        warm = scr_pool.tile([1, 2], FP16)
        nc.scalar.activation(warm[:], wts["q"][0:1, 0, 0:2], AF.Exp)

        # ---------- per-batch state ----------
        xts = [dict() for _ in range(NB)]
        qkts = [dict() for _ in range(NB)]
        vhs = [None] * NB
        stages = [None] * NB

        attn_mode = [False]   # False: prep inline (copies alt ACT/DVE); True: DVE
        alt_state = [0]

        def ccopy(dst, src):
            if not attn_mode[0]:
                alt_state[0] ^= 1
                if alt_state[0]:
                    nc.scalar.copy(dst, src)
                    return
            nc.vector.tensor_copy(dst, src)

        # ---------- work queues ----------
        # queue: low-priority prep/tail units (label, cost_ns, fn)
        # avq:   high-priority attention units
        queue = []
        avq = []
        done_labels = set()
        pe_ns = [0.0]    # attention-phase PE ns emitted
        exp_ns = [0.0]   # attention-phase exp ns emitted

        def _run(entry):
            label, cost, fn = entry
            fn()
            pe_ns[0] += cost
            if label is not None:
                done_labels.add(label)

        def pump():
            while avq and pe_ns[0] < exp_ns[0]:
                _run(avq.pop(0))
            while queue and pe_ns[0] < exp_ns[0]:
                _run(queue.pop(0))

        def drain_until(label):
            while label not in done_labels and (avq or queue):
                if avq:
                    _run(avq.pop(0))
                else:
                    _run(queue.pop(0))

        def drain_queue_until(label):
            # pop prep queue only: lets a pair gate on its projections without
            # pulling the previous pair's AV work ahead of its first S/exp
            while label not in done_labels and queue:
                _run(queue.pop(0))

        # ---------- prep units ----------
        dma_alt = [0]
        dma_count = [0]

        def emit_xdma(b, tname, tch):
            """DMA one 512-token-column chunk of host-pre-transposed x^T.
            First four chunks (b0 k/q) all go to gpsimd: the sync queue is
            busy with weight DMAs at kernel start."""
            xt = xts[b][tname]
            dma_count[0] += 1
            if dma_count[0] <= 4:
                eng = nc.gpsimd
            else:
                dma_alt[0] ^= 1
                eng = nc.gpsimd if dma_alt[0] else nc.sync
            eng.dma_start(xt[:, :, tch * E:(tch + 1) * E],
                          x_d[tname][b, :, :, tch * E:(tch + 1) * E])

        def xtile_units(b, tnames):
            return [(None, 0, lambda b=b, t=t, tch=tch: emit_xdma(b, t, tch))
                    for tch in range(NCH) for t in tnames]

        def emit_qk_proj(b, tname, fpo, tch):
            wt = wts[tname]
            xt = xts[b][tname]
            ht = qkts[b][tname]
            ps = ps_mm.tile([P, E], FP32, tag="mm")
            for epo in range(EPO):
                nc.tensor.matmul(
                    ps[:],
                    wt[:, epo, fpo * P:(fpo + 1) * P],
                    xt[:, epo, tch * E:(tch + 1) * E],
                    start=(epo == 0),
                    stop=(epo == EPO - 1),
                )
            ccopy(ht[:, fpo, tch * E:(tch + 1) * E], ps[:])

        def emit_vh(b, tt):
            vh = vhs[b]
            wt = wts["v"]
            xt = xts[b]["v"]
            ps = ps_mm.tile([P, E], FP32, tag="mm")
            for epo in range(EPO):
                nc.tensor.matmul(
                    ps[:],
                    xt[:, epo, tt * P:(tt + 1) * P],
                    wt[:, epo, :],
                    start=(epo == 0),
                    stop=(epo == EPO - 1),
                )
            ccopy(vh[:, tt, :, 0:D], ps[:].rearrange("p (h d) -> p h d", h=H))

        def prep1_units(b):
            def alloc():
                for t in ("k", "q"):
                    xts[b][t] = xt_pool.tile([P, EPO, L], FP16, tag=f"xt_{t}",
                                             name=f"xt_{b}_{t}")
                    qkts[b][t] = qk_pool.tile([P, EPO, L], FP16, tag=f"ht_{t}",
                                              name=f"ht_{b}_{t}")
            u = [(None, 0, alloc)]
            u.extend(xtile_units(b, ("k", "q")))
            for i, (t, tch) in enumerate([(t, c) for c in range(NCH)
                                          for t in ("k", "q")]):
                lbl = ("proj", b, 0) if i == 1 else None
                u.append((lbl, 900, lambda b=b, t=t, tch=tch: emit_qk_proj(b, t, 0, tch)))
            return u

        def prep2_units(b):
            def alloc():
                xts[b]["v"] = xt_pool.tile([P, EPO, L], FP16, tag="xt_v",
                                           name=f"xt_{b}_v")
                vh = vh_pool.tile([P, TT, H, 2 * D], FP16, tag="vh",
                                  name=f"vh_{b}")
                vhs[b] = vh
                nc.gpsimd.memset(vh[:, :, :, D:2 * D], 1.0)
            u = [(None, 0, alloc)]
            u.extend(xtile_units(b, ("v",)))
            for tt in range(TT):
                lbl = ("vh", b) if tt == TT - 1 else None
                u.append((lbl, 900, lambda b=b, tt=tt: emit_vh(b, tt)))
            return u

        def projf_units(b, fpo):
            u = []
            for i, (t, tch) in enumerate([(t, c) for c in range(NCH)
                                          for t in ("k", "q")]):
                lbl = ("proj", b, fpo) if i == 1 else None
                u.append((lbl, 900, lambda b=b, t=t, f=fpo, tch=tch: emit_qk_proj(b, t, f, tch)))
            return u

        # ---------- attention ----------
        def norm_from_psum(b, hpo, hh, pso, ch, on_act=False):
            """stage[head, ch] = pso[0:D] * 1/denom, denom replicated rows D:2D.
            (reciprocal_approx_* requires an SBUF source, so bounce via SBUF.)
            on_act: route the psum bounce to ScalarE (idle at pair ends)."""
            stage = stages[b]
            dns = rc_pool.tile([D, E], FP32, tag="dns")
            if on_act:
                nc.scalar.copy(dns[:], pso[D:2 * D, :])
            else:
                nc.vector.tensor_copy(dns[:], pso[D:2 * D, :])
            rc = rc_pool.tile([D, E], FP32, tag="rc")
            nc.vector.reciprocal_approx_fast(rc[:], dns[:])
            nc.vector.tensor_tensor(
                stage[D * hh:D * hh + D, hpo, ch * E:(ch + 1) * E],
                pso[0:D, :], rc[:], ALU.mult)

        def emit_outproj(b, u):
            """out^T[eoc*128:.., tch*512:..] = Wo_chunk @ stage + bias.
            Transposed form: bias is per-partition -> fused into a ScalarE
            COPY, keeping the tail chain off the (norm-busy) DVE."""
            eoc, tch = u >> 1, u & 1
            stage = stages[b]
            wt = wts["o"]
            ps = ps_mm.tile([P, E], FP32, tag="mm")
            # K=1 rank-1 update seeds the accumulator with the bias column
            nc.tensor.matmul(ps[:], bo_t[0:1, eoc * P:(eoc + 1) * P],
                             ones_row[0:1, :], start=True, stop=False)
            for epo in range(EPO):
                nc.tensor.matmul(
                    ps[:],
                    wt[:, epo, eoc * P:(eoc + 1) * P],
                    stage[:, epo, tch * E:(tch + 1) * E],
                    start=False,
                    stop=(epo == EPO - 1),
                )
            ot = o_pool.tile([P, E], FP32, tag="ot")
            nc.scalar.copy(ot[:], ps[:])
            eng = nc.gpsimd if u % 2 == 0 else nc.sync
            eng.dma_start(out_d[b, eoc * P:(eoc + 1) * P,
                                tch * E:(tch + 1) * E], ot[:])

        def attn_pair(b, hpo, last=False):
            if stages[b] is None:
                stages[b] = st_pool.tile([P, EPO, L], FP16, tag="st",
                                         name=f"stage_{b}")
            drain_queue_until(("proj", b, hpo))
            pe_ns[0] = 0.0
            exp_ns[0] = 0.0
            qht, kht = qkts[b]["q"], qkts[b]["k"]

            pso0 = [ps_av.tile([2 * D, E], FP32, tag="av",
                               name=f"pso0_{b}_{hpo}_{i}") for i in range(2)]
            # final pair: accumulate ch1 inline in the (otherwise idle) ps_mm
            # slots so the whole pair finishes at its last AV, not after two
            # deferred 8-matmul AVch1 units -> much shorter kernel tail.
            pso1 = ([ps_mm.tile([2 * D, E], FP32, tag="mm",
                                name=f"pso1_{b}_{hpo}_{i}") for i in range(2)]
                    if last else None)
            pts1 = [None] * TT
            nstart = [0, 0]
            nstart1 = [0, 0]

            def avch0(lt, pt):
                def fn():
                    vh = vhs[b]
                    for hh in range(2):
                        nstart[hh] += 1
                        nc.tensor.matmul(
                            pso0[hh][:],
                            vh[:, lt, 2 * hpo + hh, :],
                            pt[:, hh, :],
                            start=(nstart[hh] == 1),
                            stop=(nstart[hh] == TT),
                        )
                    if last:
                        pt1 = pts1[lt]
                        for hh in range(2):
                            nstart1[hh] += 1
                            nc.tensor.matmul(
                                pso1[hh][:],
                                vh[:, lt, 2 * hpo + hh, :],
                                pt1[:, hh, :],
                                start=(nstart1[hh] == 1),
                                stop=(nstart1[hh] == TT),
                            )
                return fn

            def c0_norms():
                for hh in range(2):
                    norm_from_psum(b, hpo, hh, pso0[hh], 0, on_act=True)

            def avch1(hh):
                def fn():
                    vh = vhs[b]
                    h = 2 * hpo + hh
                    pm = ps_mm.tile([2 * D, E], FP32, tag="mm")
                    for lt in range(TT):
                        nc.tensor.matmul(
                            pm[:],
                            vh[:, lt, h, :],
                            pts1[lt][:, hh, :],
                            start=(lt == 0),
                            stop=(lt == TT - 1),
                        )
                    norm_from_psum(b, hpo, hh, pm, 1)
                return fn

            delay = []
            for u in range(2 * TT):
                lt, ch = u >> 1, u & 1
                pss = ps_s.tile([P, 2, E], FP32, tag="s",
                                name=f"pss_{b}_{hpo}_{lt}_{ch}")
                for hh in range(2):
                    hoff = D * hh
                    nc.tensor.matmul(
                        pss[:, hh, :],
                        kht[hoff:hoff + D, hpo, lt * P:(lt + 1) * P],
                        qht[hoff:hoff + D, hpo, ch * E:(ch + 1) * E],
                        start=True,
                        stop=True,
                    )
                pool = p0_pool if ch == 0 else p1_pool
                pt = pool.tile([P, 2, E], FP16, tag=f"pt{ch}")
                nc.scalar.activation(pt[:].rearrange("p a b -> p (a b)"),
                                     pss[:].rearrange("p a b -> p (a b)"),
                                     AF.Exp)
                exp_ns[0] += 1400
                pe_ns[0] += 390
                if ch == 0:
                    delay.append((lt, pt))
                else:
                    pts1[lt] = pt
                if len(delay) > 1 and ("vh", b) in done_labels:
                    lt0, pt0 = delay.pop(0)
                    avq.append((None, 880 if last else 440, avch0(lt0, pt0)))
                if u == 6:
                    # hard deadline for the previous pair's AV carryover:
                    # everything the pt0/pt1/ps_av rings may WAR-wait on must
                    # be emitted before this pair reuses those slots
                    while avq:
                        _run(avq.pop(0))
                if last:
                    while avq and pe_ns[0] < exp_ns[0]:
                        _run(avq.pop(0))
                else:
                    pump()
            drain_until(("vh", b))
            for lt0, pt0 in delay:
                avq.append((None, 427, avch0(lt0, pt0)))
            delay.clear()
            avq.append((None, 100, c0_norms))
            avq.append((None, 1750, avch1(0)))
            avq.append((("avdone", b, hpo), 1750, avch1(1)))

        # =================== schedule ===================
        for u in prep1_units(0):
            _run(u)
        queue.append((None, 0, lambda: emit_wt("v")))
        queue.extend(prep2_units(0))
        for fpo in range(1, EPO):
            queue.extend(projf_units(0, fpo))
        queue.append((None, 0, lambda: emit_wt("o")))
        queue.extend(prep1_units(1))
        queue.extend(projf_units(1, 0))
        queue.extend(prep2_units(1))
        for fpo in range(1, EPO):
            queue.extend(projf_units(1, fpo))

        attn_mode[0] = True
        pe_ns[0] = 0.0
        exp_ns[0] = 0.0
        for b in range(NB):
            for hpo in range(H // 2):
                attn_pair(b, hpo, last=(b == NB - 1 and hpo == H // 2 - 1))
                if hpo == H // 2 - 1:
                    if b == NB - 1:
                        def gate0(b=b):
                            drain_until(("avdone0", b))
                        queue.append((None, 0, gate0))
                        queue.extend((None, 900, lambda b=b, u=u: emit_outproj(b, u))
                                     for u in range(0, 2 * EPO, 2))
                        def gate(b=b):
                            drain_until(("avdone", b, H // 2 - 1))
                        queue.append((None, 0, gate))
                        queue.extend((None, 900, lambda b=b, u=u: emit_outproj(b, u))
                                     for u in range(1, 2 * EPO, 2))
                    else:
                        def gate(b=b):
                            drain_until(("avdone", b, H // 2 - 1))
                        queue.append((None, 0, gate))
                        queue.extend((None, 900, lambda b=b, u=u: emit_outproj(b, u))
                                     for u in range(2 * EPO))
        while avq or queue:
            if avq:
                _run(avq.pop(0))
            else:
                _run(queue.pop(0))

    nc.compile()
    return nc


_COMPILED = None


def _get_compiled():
    global _COMPILED
    if _COMPILED is None:
        _COMPILED = build()
    return _COMPILED


def host_weights(Wq, Wk, Wv, Wo):
    """W [f, e] -> wt [ei, eo, f] fp16 (== W^T in the kernel's layout);
    wq pre-scaled by 1/sqrt(D)."""
    def tr(w, scale=1.0):
        wt = (w.T.astype(np.float64) * scale).astype(np.float32)
        return np.ascontiguousarray(
            wt.reshape(EPO, P, E).transpose(1, 0, 2)).astype(np.float16)
    return {
        "Wt_q": tr(Wq, 1.0 / np.sqrt(D)),
        "Wt_k": tr(Wk),
        "Wt_v": tr(Wv),
        "Wt_o": tr(Wo),
    }


def host_xt(x16):
    """[n, L, E] fp16 -> [n, P, EPO, L]: xt[b, p, epo, t] = x[b, t, epo*P+p]"""
    n = x16.shape[0]
    xt = x16.transpose(0, 2, 1).reshape(n, EPO, P, L).transpose(0, 2, 1, 3)
    return np.ascontiguousarray(xt)


def make_in_maps(q, k, v, Wq, Wk, Wv, Wo, bo, n_cores=8):
    wts = host_weights(np.asarray(Wq, np.float32), np.asarray(Wk, np.float32),
                       np.asarray(Wv, np.float32), np.asarray(Wo, np.float32))
    bo_row = np.ascontiguousarray(
        np.asarray(bo, np.float32).reshape(1, E).astype(np.float16))
    qT = host_xt(np.asarray(q, np.float32).astype(np.float16))
    kT = host_xt(np.asarray(k, np.float32).astype(np.float16))
    vT = host_xt(np.asarray(v, np.float32).astype(np.float16))
    in_maps = []
    for c in range(n_cores):
        in_maps.append({
            "qT": np.ascontiguousarray(qT[c * NB:(c + 1) * NB]),
            "kT": np.ascontiguousarray(kT[c * NB:(c + 1) * NB]),
            "vT": np.ascontiguousarray(vT[c * NB:(c + 1) * NB]),
            **wts,
            "bo_row": bo_row,
        })
    return in_maps


def kernel(q, k, v, Wq, Wk, Wv, Wo, bo):
    nc = _get_compiled()
    in_maps = make_in_maps(q, k, v, Wq, Wk, Wv, Wo, bo)
    from concourse.bass_utils import run_bass_kernel_spmd
    res = run_bass_kernel_spmd(nc, in_maps, core_ids=list(range(8)))
    outT = np.concatenate([res.results[c]["outT"] for c in range(8)], axis=0)
    return np.ascontiguousarray(outT.transpose(0, 2, 1)).astype(np.float32)


# revision 24
# speedup vs baseline: 1.1985x; 1.1985x over previous
"""Trainium2 Bass kernel for ConvMultiHeadAttention (N=16, L=1024, E=512, H=8).

Data-parallel over batch: 8 NeuronCores x 2 batches each.

v2 design (vs v1 baseline at 313us):
  - Host marshalling: q/k/v pre-cast to fp16 (halves input DMA bytes, kills
    48 device CAST instrs), weights pre-transposed + pre-cast + wq pre-scaled
    on host (kills 64 PE transposes + 16 psum copies + 4MB of DMA).
  - Attention pipelined at (lt, ch) granularity: S pair matmuls write a
    [P, 2, 512] fp32 psum tile, ONE exp instr per unit covers both heads.
    AVch0 accumulation lags ~1 lt behind exp via an explicit high-priority
    AV queue; AVch1 re-reads kept pts as coarse filler units.
  - vh stationary is ones-extended to 128 columns: the AV matmul replicates
    the softmax denominator across psum partitions 64..127 for free (M does
    not affect matmul time). reciprocal_approx_fast reads it straight from
    PSUM partition-parallel, and a fused scalar_tensor_tensor writes the
    normalized stage in one DVE op. No [1,512] denominator copies, no
    selector matmuls, no cross-pair recip gating.
  - Integral-controller pump: tracks emitted attention-PE-ns vs exp-ns and
    feeds prep/proj/outproj filler units into the PE stream at the deficit
    rate, so the queue lasts through BOTH batches' attention phases.
  - Exp activation table preloaded by a dummy exp during the DMA fill.
"""

import numpy as np
import concourse.bass as bass
import concourse.mybir as mybir
import concourse.tile as tile
from contextlib import ExitStack
from concourse import bacc

P = 128
L = 1024
E = 512
H = 8
D = 64
NB = 2            # batches per core
TT = L // P       # 8 token tiles per batch
EPO = E // P      # 4 e-subtiles
NCH = L // E      # 2 q-chunks
FP32 = mybir.dt.float32
FP16 = mybir.dt.float16
AF = mybir.ActivationFunctionType
ALU = mybir.AluOpType


def build(debug=False):
    nc = bacc.Bacc("TRN2", target_bir_lowering=False, debug=debug)
    q_d = nc.dram_tensor("qT", [NB, P, EPO, L], FP16, kind="ExternalInput").ap()
    k_d = nc.dram_tensor("kT", [NB, P, EPO, L], FP16, kind="ExternalInput").ap()
    v_d = nc.dram_tensor("vT", [NB, P, EPO, L], FP16, kind="ExternalInput").ap()
    wt_d = {
        w: nc.dram_tensor(f"Wt_{w}", [P, EPO, E], FP16, kind="ExternalInput").ap()
        for w in ("q", "k", "v", "o")
    }
    bo_d = nc.dram_tensor("bo_row", [1, E], FP16, kind="ExternalInput").ap()
    out_d = nc.dram_tensor("outT", [NB, E, L], FP32, kind="ExternalOutput").ap()
    x_d = {"q": q_d, "k": k_d, "v": v_d}

    with tile.TileContext(nc) as tc, ExitStack() as ctx:
        consts = ctx.enter_context(tc.tile_pool(name="consts", bufs=1))
        wt_pool = ctx.enter_context(tc.tile_pool(name="wt", bufs=1))
        xt_pool = ctx.enter_context(tc.tile_pool(name="xt", bufs=2))
        qk_pool = ctx.enter_context(tc.tile_pool(name="qk", bufs=2))
        vh_pool = ctx.enter_context(tc.tile_pool(name="vh", bufs=2))
        st_pool = ctx.enter_context(tc.tile_pool(name="st", bufs=2))
        p0_pool = ctx.enter_context(tc.tile_pool(name="p0", bufs=8))
        p1_pool = ctx.enter_context(tc.tile_pool(name="p1", bufs=12))
        rc_pool = ctx.enter_context(tc.tile_pool(name="rc", bufs=2))
        o_pool = ctx.enter_context(tc.tile_pool(name="oo", bufs=4))
        scr_pool = ctx.enter_context(tc.tile_pool(name="scr", bufs=1))
        ps_s = ctx.enter_context(tc.tile_pool(name="pss", bufs=2, space="PSUM"))
        ps_av = ctx.enter_context(tc.tile_pool(name="psav", bufs=2, space="PSUM"))
        ps_mm = ctx.enter_context(tc.tile_pool(name="psmm", bufs=2, space="PSUM"))

        # ---- constants (sync queue) ----
        wts = {}
        for w in ("k", "q"):
            wt = wt_pool.tile([P, EPO, E], FP16, tag=f"wt_{w}", name=f"wt_{w}")
            nc.sync.dma_start(wt[:], wt_d[w])
            wts[w] = wt

        def emit_wt(w):
            wt = wt_pool.tile([P, EPO, E], FP16, tag=f"wt_{w}", name=f"wt_{w}")
            nc.sync.dma_start(wt[:], wt_d[w])
            wts[w] = wt

        bo_t = consts.tile([1, E], FP16)
        nc.sync.dma_start(bo_t[:], bo_d)
        ones_row = consts.tile([1, E], FP16)
        nc.gpsimd.memset(ones_row[:], 1.0)

        # preload the Exp table while DMAs stream in
        warm = scr_pool.tile([1, 2], FP16)
        nc.scalar.activation(warm[:], wts["q"][0:1, 0, 0:2], AF.Exp)

        # ---------- per-batch state ----------
        xts = [dict() for _ in range(NB)]
        qkts = [dict() for _ in range(NB)]
        vhs = [None] * NB
        stages = [None] * NB

        attn_mode = [False]   # False: prep inline (copies alt ACT/DVE); True: DVE
        alt_state = [0]

        def ccopy(dst, src):
            if not attn_mode[0]:
                alt_state[0] ^= 1
                if alt_state[0]:
                    nc.scalar.copy(dst, src)
                    return
            nc.vector.tensor_copy(dst, src)

        # ---------- work queues ----------
        # queue: low-priority prep/tail units (label, cost_ns, fn)
        # avq:   high-priority attention units
        queue = []
        avq = []
        done_labels = set()
        pe_ns = [0.0]    # attention-phase PE ns emitted
        exp_ns = [0.0]   # attention-phase exp ns emitted

        def _run(entry):
            label, cost, fn = entry
            fn()
            pe_ns[0] += cost
            if label is not None:
                done_labels.add(label)

        def pump():
            while avq and pe_ns[0] < exp_ns[0]:
                _run(avq.pop(0))
            while queue and pe_ns[0] < exp_ns[0]:
                _run(queue.pop(0))

        def drain_until(label):
            while label not in done_labels and (avq or queue):
                if avq:
                    _run(avq.pop(0))
                else:
                    _run(queue.pop(0))

        def drain_queue_until(label):
            # pop prep queue only: lets a pair gate on its projections without
            # pulling the previous pair's AV work ahead of its first S/exp
            while label not in done_labels and queue:
                _run(queue.pop(0))

        # ---------- prep units ----------
        dma_alt = [0]
        dma_count = [0]

        def emit_xdma(b, tname, tch):
            """DMA one 512-token-column chunk of host-pre-transposed x^T.
            First four chunks (b0 k/q) all go to gpsimd: the sync queue is
            busy with weight DMAs at kernel start."""
            xt = xts[b][tname]
            dma_count[0] += 1
            if dma_count[0] <= 4:
                eng = nc.gpsimd
            else:
                dma_alt[0] ^= 1
                eng = nc.gpsimd if dma_alt[0] else nc.sync
            eng.dma_start(xt[:, :, tch * E:(tch + 1) * E],
                          x_d[tname][b, :, :, tch * E:(tch + 1) * E])

        def xtile_units(b, tnames):
            return [(None, 0, lambda b=b, t=t, tch=tch: emit_xdma(b, t, tch))
                    for tch in range(NCH) for t in tnames]

        def emit_qk_proj(b, tname, fpo, tch):
            wt = wts[tname]
            xt = xts[b][tname]
            ht = qkts[b][tname]
            ps = ps_mm.tile([P, E], FP32, tag="mm")
            for epo in range(EPO):
                nc.tensor.matmul(
                    ps[:],
                    wt[:, epo, fpo * P:(fpo + 1) * P],
                    xt[:, epo, tch * E:(tch + 1) * E],
                    start=(epo == 0),
                    stop=(epo == EPO - 1),
                )
            ccopy(ht[:, fpo, tch * E:(tch + 1) * E], ps[:])

        def emit_vh(b, tt):
            vh = vhs[b]
            wt = wts["v"]
            xt = xts[b]["v"]
            ps = ps_mm.tile([P, E], FP32, tag="mm")
            for epo in range(EPO):
                nc.tensor.matmul(
                    ps[:],
                    xt[:, epo, tt * P:(tt + 1) * P],
                    wt[:, epo, :],
                    start=(epo == 0),
                    stop=(epo == EPO - 1),
                )
            ccopy(vh[:, tt, :, 0:D], ps[:].rearrange("p (h d) -> p h d", h=H))

        def prep1_units(b):
            def alloc():
                for t in ("k", "q"):
                    xts[b][t] = xt_pool.tile([P, EPO, L], FP16, tag=f"xt_{t}",
                                             name=f"xt_{b}_{t}")
                    qkts[b][t] = qk_pool.tile([P, EPO, L], FP16, tag=f"ht_{t}",
                                              name=f"ht_{b}_{t}")
            u = [(None, 0, alloc)]
            u.extend(xtile_units(b, ("k", "q")))
            for i, (t, tch) in enumerate([(t, c) for c in range(NCH)
                                          for t in ("k", "q")]):
                lbl = ("proj", b, 0) if i == 1 else None
                u.append((lbl, 900, lambda b=b, t=t, tch=tch: emit_qk_proj(b, t, 0, tch)))
            return u

        def prep2_units(b):
            def alloc():
                xts[b]["v"] = xt_pool.tile([P, EPO, L], FP16, tag="xt_v",
                                           name=f"xt_{b}_v")
                vh = vh_pool.tile([P, TT, H, 2 * D], FP16, tag="vh",
                                  name=f"vh_{b}")
                vhs[b] = vh
                nc.gpsimd.memset(vh[:, :, :, D:2 * D], 1.0)
            u = [(None, 0, alloc)]
            u.extend(xtile_units(b, ("v",)))
            for tt in range(TT):
                lbl = ("vh", b) if tt == TT - 1 else None
                u.append((lbl, 900, lambda b=b, tt=tt: emit_vh(b, tt)))
            return u

        def projf_units(b, fpo):
            u = []
            for i, (t, tch) in enumerate([(t, c) for c in range(NCH)
                                          for t in ("k", "q")]):
                lbl = ("proj", b, fpo) if i == 1 else None
                u.append((lbl, 900, lambda b=b, t=t, f=fpo, tch=tch: emit_qk_proj(b, t, f, tch)))
            return u

        # ---------- attention ----------
        def norm_from_psum(b, hpo, hh, pso, ch, on_act=False):
            """stage[head, ch] = pso[0:D] * 1/denom, denom replicated rows D:2D.
            (reciprocal_approx_* requires an SBUF source, so bounce via SBUF.)
            on_act: route the psum bounce to ScalarE (idle at pair ends)."""
            stage = stages[b]
            dns = rc_pool.tile([D, E], FP32, tag="dns")
            if on_act:
                nc.scalar.copy(dns[:], pso[D:2 * D, :])
            else:
                nc.vector.tensor_copy(dns[:], pso[D:2 * D, :])
            rc = rc_pool.tile([D, E], FP32, tag="rc")
            nc.vector.reciprocal_approx_fast(rc[:], dns[:])
            nc.vector.tensor_tensor(
                stage[D * hh:D * hh + D, hpo, ch * E:(ch + 1) * E],
                pso[0:D, :], rc[:], ALU.mult)

        def emit_outproj(b, u):
            """out^T[eoc*128:.., tch*512:..] = Wo_chunk @ stage + bias.
            Transposed form: bias is per-partition -> fused into a ScalarE
            COPY, keeping the tail chain off the (norm-busy) DVE."""
            eoc, tch = u >> 1, u & 1
            stage = stages[b]
            wt = wts["o"]
            ps = ps_mm.tile([P, E], FP32, tag="mm")
            # K=1 rank-1 update seeds the accumulator with the bias column
            nc.tensor.matmul(ps[:], bo_t[0:1, eoc * P:(eoc + 1) * P],
                             ones_row[0:1, :], start=True, stop=False)
            for epo in range(EPO):
                nc.tensor.matmul(
                    ps[:],
                    wt[:, epo, eoc * P:(eoc + 1) * P],
                    stage[:, epo, tch * E:(tch + 1) * E],
                    start=False,
                    stop=(epo == EPO - 1),
                )
            ot = o_pool.tile([P, E], FP32, tag="ot")
            if b == NB - 1:
                nc.scalar.copy(ot[:], ps[:])
            else:
                nc.vector.tensor_copy(ot[:], ps[:])
            eng = nc.gpsimd if u % 2 == 0 else nc.sync
            eng.dma_start(out_d[b, eoc * P:(eoc + 1) * P,
                                tch * E:(tch + 1) * E], ot[:])

        def attn_pair(b, hpo, last=False):
            if stages[b] is None:
                stages[b] = st_pool.tile([P, EPO, L], FP16, tag="st",
                                         name=f"stage_{b}")
            drain_queue_until(("proj", b, hpo))
            pe_ns[0] = 0.0
            exp_ns[0] = 0.0
            qht, kht = qkts[b]["q"], qkts[b]["k"]

            pso0 = [ps_av.tile([2 * D, E], FP32, tag="av",
                               name=f"pso0_{b}_{hpo}_{i}") for i in range(2)]
            # final pair: accumulate ch1 inline in the (otherwise idle) ps_mm
            # slots so the whole pair finishes at its last AV, not after two
            # deferred 8-matmul AVch1 units -> much shorter kernel tail.
            pso1 = ([ps_mm.tile([2 * D, E], FP32, tag="mm",
                                name=f"pso1_{b}_{hpo}_{i}") for i in range(2)]
                    if last else None)
            pts1 = [None] * TT
            nstart = [0, 0]
            nstart1 = [0, 0]

            def avch0(lt, pt):
                def fn():
                    vh = vhs[b]
                    for hh in range(2):
                        nstart[hh] += 1
                        nc.tensor.matmul(
                            pso0[hh][:],
                            vh[:, lt, 2 * hpo + hh, :],
                            pt[:, hh, :],
                            start=(nstart[hh] == 1),
                            stop=(nstart[hh] == TT),
                        )
                    if last:
                        pt1 = pts1[lt]
                        for hh in range(2):
                            nstart1[hh] += 1
                            nc.tensor.matmul(
                                pso1[hh][:],
                                vh[:, lt, 2 * hpo + hh, :],
                                pt1[:, hh, :],
                                start=(nstart1[hh] == 1),
                                stop=(nstart1[hh] == TT),
                            )
                return fn

            def c0_norms():
                for hh in range(2):
                    norm_from_psum(b, hpo, hh, pso0[hh], 0)

            def avch1(hh):
                def fn():
                    vh = vhs[b]
                    h = 2 * hpo + hh
                    pm = ps_mm.tile([2 * D, E], FP32, tag="mm")
                    for lt in range(TT):
                        nc.tensor.matmul(
                            pm[:],
                            vh[:, lt, h, :],
                            pts1[lt][:, hh, :],
                            start=(lt == 0),
                            stop=(lt == TT - 1),
                        )
                    norm_from_psum(b, hpo, hh, pm, 1)
                return fn

            delay = []
            for u in range(2 * TT):
                lt, ch = u >> 1, u & 1
                pss = ps_s.tile([P, 2, E], FP32, tag="s",
                                name=f"pss_{b}_{hpo}_{lt}_{ch}")
                for hh in range(2):
                    hoff = D * hh
                    nc.tensor.matmul(
                        pss[:, hh, :],
                        kht[hoff:hoff + D, hpo, lt * P:(lt + 1) * P],
                        qht[hoff:hoff + D, hpo, ch * E:(ch + 1) * E],
                        start=True,
                        stop=True,
                    )
                pool = p0_pool if ch == 0 else p1_pool
                pt = pool.tile([P, 2, E], FP16, tag=f"pt{ch}")
                nc.scalar.activation(pt[:].rearrange("p a b -> p (a b)"),
                                     pss[:].rearrange("p a b -> p (a b)"),
                                     AF.Exp)
                exp_ns[0] += 1400
                pe_ns[0] += 390
                if ch == 0:
                    delay.append((lt, pt))
                else:
                    pts1[lt] = pt
                if len(delay) > 1 and ("vh", b) in done_labels:
                    lt0, pt0 = delay.pop(0)
                    avq.append((None, 880 if last else 440, avch0(lt0, pt0)))
                if u == 6:
                    # hard deadline for the previous pair's AV carryover:
                    # everything the pt0/pt1/ps_av rings may WAR-wait on must
                    # be emitted before this pair reuses those slots
                    while avq:
                        _run(avq.pop(0))
                if last:
                    while avq and pe_ns[0] < exp_ns[0]:
                        _run(avq.pop(0))
                else:
                    pump()
            drain_until(("vh", b))
            for lt0, pt0 in delay:
                avq.append((None, 427, avch0(lt0, pt0)))
            delay.clear()
            avq.append((None, 100, c0_norms))
            avq.append((None, 1750, avch1(0)))
            avq.append((("avdone", b, hpo), 1750, avch1(1)))

        # =================== schedule ===================
        for u in prep1_units(0):
            _run(u)
        queue.append((None, 0, lambda: emit_wt("v")))
        queue.extend(prep2_units(0))
        for fpo in range(1, EPO):
            queue.extend(projf_units(0, fpo))
        queue.append((None, 0, lambda: emit_wt("o")))
        queue.extend(prep1_units(1))
        queue.extend(projf_units(1, 0))
        queue.extend(prep2_units(1))
        for fpo in range(1, EPO):
            queue.extend(projf_units(1, fpo))

        attn_mode[0] = True
        pe_ns[0] = 0.0
        exp_ns[0] = 0.0
        for b in range(NB):
            for hpo in range(H // 2):
                attn_pair(b, hpo, last=(b == NB - 1 and hpo == H // 2 - 1))
                if hpo == H // 2 - 1:
                    if b == NB - 1:
                        def gate0(b=b):
                            drain_until(("avdone0", b))
                        queue.append((None, 0, gate0))
                        queue.extend((None, 900, lambda b=b, u=u: emit_outproj(b, u))
                                     for u in range(0, 2 * EPO, 2))
                        def gate(b=b):
                            drain_until(("avdone", b, H // 2 - 1))
                        queue.append((None, 0, gate))
                        queue.extend((None, 900, lambda b=b, u=u: emit_outproj(b, u))
                                     for u in range(1, 2 * EPO, 2))
                    else:
                        def gate(b=b):
                            drain_until(("avdone", b, H // 2 - 1))
                        queue.append((None, 0, gate))
                        queue.extend((None, 900, lambda b=b, u=u: emit_outproj(b, u))
                                     for u in range(2 * EPO))
        while avq or queue:
            if avq:
                _run(avq.pop(0))
            else:
                _run(queue.pop(0))

    nc.compile()
    return nc


_COMPILED = None


def _get_compiled():
    global _COMPILED
    if _COMPILED is None:
        _COMPILED = build()
    return _COMPILED


def host_weights(Wq, Wk, Wv, Wo):
    """W [f, e] -> wt [ei, eo, f] fp16 (== W^T in the kernel's layout);
    wq pre-scaled by 1/sqrt(D)."""
    def tr(w, scale=1.0):
        wt = (w.T.astype(np.float64) * scale).astype(np.float32)
        return np.ascontiguousarray(
            wt.reshape(EPO, P, E).transpose(1, 0, 2)).astype(np.float16)
    return {
        "Wt_q": tr(Wq, 1.0 / np.sqrt(D)),
        "Wt_k": tr(Wk),
        "Wt_v": tr(Wv),
        "Wt_o": tr(Wo),
    }


def host_xt(x16):
    """[n, L, E] fp16 -> [n, P, EPO, L]: xt[b, p, epo, t] = x[b, t, epo*P+p]"""
    n = x16.shape[0]
    xt = x16.transpose(0, 2, 1).reshape(n, EPO, P, L).transpose(0, 2, 1, 3)
    return np.ascontiguousarray(xt)


def make_in_maps(q, k, v, Wq, Wk, Wv, Wo, bo, n_cores=8):
    wts = host_weights(np.asarray(Wq, np.float32), np.asarray(Wk, np.float32),
                       np.asarray(Wv, np.float32), np.asarray(Wo, np.float32))
    bo_row = np.ascontiguousarray(
        np.asarray(bo, np.float32).reshape(1, E).astype(np.float16))
    qT = host_xt(np.asarray(q, np.float32).astype(np.float16))
    kT = host_xt(np.asarray(k, np.float32).astype(np.float16))
    vT = host_xt(np.asarray(v, np.float32).astype(np.float16))
    in_maps = []
    for c in range(n_cores):
        in_maps.append({
            "qT": np.ascontiguousarray(qT[c * NB:(c + 1) * NB]),
            "kT": np.ascontiguousarray(kT[c * NB:(c + 1) * NB]),
            "vT": np.ascontiguousarray(vT[c * NB:(c + 1) * NB]),
            **wts,
            "bo_row": bo_row,
        })
    return in_maps


def kernel(q, k, v, Wq, Wk, Wv, Wo, bo):
    nc = _get_compiled()
    in_maps = make_in_maps(q, k, v, Wq, Wk, Wv, Wo, bo)
    from concourse.bass_utils import run_bass_kernel_spmd
    res = run_bass_kernel_spmd(nc, in_maps, core_ids=list(range(8)))
    outT = np.concatenate([res.results[c]["outT"] for c in range(8)], axis=0)
    return np.ascontiguousarray(outT.transpose(0, 2, 1)).astype(np.float32)
